# revision 1
# baseline (speedup 1.0000x reference)
"""Self-contained Trainium2 Bass kernel for MultiHeadAttention.

Problem: B=2, S=2048, D=1024, H=16, hd=64, with the reference's
masked_fill(mask==0, -1e-09) quirk: masked scores become ~0.0, so
exp(masked) == 1.0 in fp32 and every key position participates in the
softmax denominator. Fully-masked key blocks therefore contribute a
block-constant suffix sum of V rows, added via cheap rank-1-style
matmuls instead of full score/attn matmuls.

Sharding: 8 cores = 2 batches x 4 head-groups (4 heads per core).
Each core computes a partial [S, D] output (its 4 heads pushed through
the O-projection); the host sums the 4 partials per batch and adds bo.

Layouts (per core, all matmul operands at partition base 0):
  qt  [128, pair, S]   q^T, two heads stacked on partitions (d dims)
  ktz [128, head, S]   k^T zero-padded: even heads live on partitions
                       0-63 (64-127 zero), odd heads on 64-127 — the
                       scores matmul is then a plain K=128 matmul
                       against the pair-stacked qt.
  v2  [128, head, kj, 65]  V blocks with an appended ones column
                       (produces the softmax denominator for free).
  scores^T [sk, sq] in PSUM -> exp on ScalarE -> bf16 tiles ->
  attnU^T [65, sq] accumulated with V2 stationary (N=512 moving), so
  no transposes are needed before the O-projection; rowsum = row 64.
  Reciprocal of the [1, 512] rowsum rows via exp(-ln(r)) on ScalarE,
  replicated across partitions by a tiny SBUF->SBUF DMA.
"""

import numpy as np
import ml_dtypes

import concourse.bass as bass
import concourse.bacc as bacc
import concourse.tile as tile
import concourse.mybir as mybir
from concourse.bass_utils import run_bass_kernel_spmd

BF16 = mybir.dt.bfloat16
F32 = mybir.dt.float32
NPBF16 = ml_dtypes.bfloat16
AF = mybir.ActivationFunctionType

B = 2
S = 2048
D = 1024
H = 16
HD = 64
NCORES = 8
HPC = 4            # heads per core
NPAIRS = 2         # head pairs per core
NQ = S // 128      # 16 query/key blocks of 128
QCH = 512          # sq chunk width
NCH = S // QCH     # 4 chunks
KT = D // 128      # 8 contraction tiles for projections


def _emit(tc: tile.TileContext, io: dict):
    nc = tc.nc

    persist = tc.alloc_tile_pool(name="persist", bufs=1)

    # ---- constants ----
    ones128 = persist.tile([128, 128], BF16, name="ones128")
    nc.gpsimd.memset(ones128, 1.0)
    onesrow = persist.tile([1, 512], BF16, name="onesrow")
    nc.gpsimd.memset(onesrow, 1.0)

    # ---- persistent SBUF arrays ----
    qt = persist.tile([128, NPAIRS, S], BF16, name="qt")
    ktz = persist.tile([128, HPC, S], BF16, name="ktz")
    v2 = persist.tile([128, HPC, NQ, 65], BF16, name="v2")
    fs = persist.tile([128, HPC, NQ, 65], BF16, name="fs")
    att = persist.tile([128, NPAIRS, S], BF16, name="att")

    qts = persist.tile([128, KT, S], BF16, name="qts")
    kts = persist.tile([128, KT, S], BF16, name="kts")
    vts = persist.tile([128, KT, S], BF16, name="vts")
    wqt = persist.tile([128, KT, 256], BF16, name="wqt")
    wkt = persist.tile([128, KT, 256], BF16, name="wkt")
    wvt = persist.tile([128, KT, 256], BF16, name="wvt")
    wot = persist.tile([128, NPAIRS, D], BF16, name="wot")
    bq_sb = persist.tile([1, 256], BF16, name="bq_sb")
    bk_sb = persist.tile([1, 256], BF16, name="bk_sb")
    bv_sb = persist.tile([1, 256], BF16, name="bv_sb")

    # DMA descriptor issue costs ~600ns on the issuing engine: spread the
    # triggers over SP / GpSimd / ACT, V first (v-proj runs first).
    dma = nc.sync
    dma2 = nc.gpsimd
    nc.scalar.dma_start(bv_sb, io["bv"])
    nc.scalar.dma_start(bq_sb, io["bq"])
    nc.scalar.dma_start(bk_sb, io["bk"])
    for t in range(KT):
        dma.dma_start(wvt[:, t, :], io["WvT"][t * 128:(t + 1) * 128, :])
        dma.dma_start(vts[:, t, 0:QCH], io["VT"][t * 128:(t + 1) * 128, 0:QCH])
        dma2.dma_start(wqt[:, t, :], io["WqT"][t * 128:(t + 1) * 128, :])
        dma2.dma_start(wkt[:, t, :], io["WkT"][t * 128:(t + 1) * 128, :])
        nc.scalar.dma_start(qts[:, t, 0:QCH],
                            io["QT"][t * 128:(t + 1) * 128, 0:QCH])
    for t in range(KT):
        dma2.dma_start(kts[:, t, 0:QCH], io["KT"][t * 128:(t + 1) * 128, 0:QCH])
    for t in range(KT):
        dma.dma_start(vts[:, t, QCH:], io["VT"][t * 128:(t + 1) * 128, QCH:])
        nc.scalar.dma_start(qts[:, t, QCH:],
                            io["QT"][t * 128:(t + 1) * 128, QCH:])
        dma2.dma_start(kts[:, t, QCH:], io["KT"][t * 128:(t + 1) * 128, QCH:])
    for p in range(NPAIRS):
        nc.scalar.dma_start(wot[:, p, :], io["WoT"][p * 128:(p + 1) * 128, :])

    for h in range(HPC):  # zero the unused half of each ktz head
        half = slice(64, 128) if h % 2 == 0 else slice(0, 64)
        nc.vector.memset(ktz[half, h, :], 0.0)
    nc.gpsimd.memset(v2[:, :, :, 64:65], 1.0)  # ones column

    pb_s = tc.alloc_tile_pool(name="pb_scores", bufs=2, space="PSUM")
    pb_a = tc.alloc_tile_pool(name="pb_attnu", bufs=2, space="PSUM")
    pb_e = tc.alloc_tile_pool(name="pb_exp", bufs=7)
    pb_r = tc.alloc_tile_pool(name="pb_recip", bufs=2)

    def vproj(c):
        """V projection for key blocks 4c..4c+3 -> v2 tiles."""
        for st in range(4 * c, 4 * c + 4):
            psv_t = pb_s.tile([128, 2, QCH], F32, tag="sps", name=f"ps_v{st}")
            ps_v = psv_t[:, 0, 0:256]
            for t in range(KT):
                nc.tensor.matmul(ps_v, vts[:, t, st * 128:(st + 1) * 128],
                                 wvt[:, t, :], start=(t == 0), stop=False)
            nc.tensor.matmul(ps_v, ones128[0:1, :], bv_sb,
                             start=False, stop=True)  # + bv rank-1
            for h in range(HPC):
                nc.vector.tensor_copy(v2[:, h, st, 0:64],
                                      ps_v[:, h * 64:(h + 1) * 64])

    def qproj_unit(c, p):
        sq = slice(c * QCH, (c + 1) * QCH)
        psq_t = pb_s.tile([128, 2, QCH], F32, tag="sps", name=f"ps_q{p}_{c}")
        ps_q = psq_t[:, 0, :]
        for t in range(KT):
            nc.tensor.matmul(ps_q, wqt[:, t, p * 128:(p + 1) * 128],
                             qts[:, t, sq], start=(t == 0), stop=False)
        nc.tensor.matmul(ps_q, bq_sb[0:1, p * 128:(p + 1) * 128],
                         onesrow, start=False, stop=True)  # + bq rank-1
        nc.vector.tensor_copy(qt[:, p, sq], ps_q)

    def kproj_unit(c, p):
        sq = slice(c * QCH, (c + 1) * QCH)
        psk_t = pb_s.tile([128, 2, QCH], F32, tag="sps", name=f"ps_k{p}_{c}")
        ps_k = psk_t[:, 0, :]
        for t in range(KT):
            nc.tensor.matmul(ps_k, wkt[:, t, p * 128:(p + 1) * 128],
                             kts[:, t, sq], start=(t == 0), stop=False)
        nc.tensor.matmul(ps_k, bk_sb[0:1, p * 128:(p + 1) * 128],
                         onesrow, start=False, stop=True)  # + bk rank-1
        nc.vector.tensor_copy(ktz[0:64, 2 * p, sq], ps_k[0:64, :])
        nc.vector.tensor_copy(ktz[64:128, 2 * p + 1, sq], ps_k[64:128, :])

    def qkproj(c):
        for p in range(NPAIRS):
            qproj_unit(c, p)
            kproj_unit(c, p)

    def qk_fillers(c):
        return [lambda p=p, f=f: f(c, p)
                for p in range(NPAIRS) for f in (qproj_unit, kproj_unit)]

    def folded_suffixes():
        nc.vector.memset(fs[:, :, NQ - 1, :], 0.0)
        for h in range(HPC):
            for q in range(NQ - 2, -1, -1):
                nc.vector.tensor_add(fs[:, h, q, :], fs[:, h, q + 1, :],
                                     v2[:, h, q + 1, :])

    aups_tiles = {}

    def chunk_loop(c, fillers=()):
        """scores -> exp -> attnU^T accumulation for chunk c, both pairs.
        One filler (an independent PE work unit) is emitted after each kj
        iteration to keep the PE stream busy while ACT/DVE chains drain."""
        fillers = list(fillers)
        nkj = 2 * (4 * c + 4)
        per = max(1, (len(fillers) + nkj - 1) // nkj) if fillers else 0
        for p in range(NPAIRS):
            aups = pb_a.tile([65, 2, QCH], F32, tag="aups", name=f"aups{p}_{c}")
            aups_tiles[(p, c)] = aups
            for kj in range(4 * c + 4):
                c0 = max(kj - 4 * c, 0) * 128   # first valid col in chunk
                sps = pb_s.tile([128, 2, QCH], F32, tag="sps",
                                name=f"sps{p}_{c}_{kj}")
                for hl in range(2):
                    nc.tensor.matmul(
                        sps[:, hl, c0:QCH],
                        ktz[:, 2 * p + hl, kj * 128:(kj + 1) * 128],
                        qt[:, p, c * QCH + c0:(c + 1) * QCH],
                        start=True, stop=True)
                ext = pb_e.tile([128, 2, QCH], BF16, tag="ext",
                                name=f"ext{p}_{c}_{kj}")
                nc.scalar.activation(ext[:, :, c0:QCH], sps[:, :, c0:QCH],
                                     AF.Exp, scale=0.125)
                if kj >= 4 * c:  # diagonal block: masked exp entries -> 1.0
                    for hl in range(2):
                        nc.gpsimd.affine_select(
                            out=ext[:, hl, c0:c0 + 128],
                            in_=ext[:, hl, c0:c0 + 128],
                            compare_op=mybir.AluOpType.is_ge,
                            fill=1.0, base=0,
                            pattern=[[1, 128]], channel_multiplier=-1)
                for hl in range(2):
                    # masked cols < c0 get their (block-constant)
                    # contribution from the early FS matmuls below
                    nc.tensor.matmul(
                        aups[:, hl, c0:QCH],
                        v2[:, 2 * p + hl, kj, :],
                        ext[:, hl, c0:QCH],
                        start=(kj == 0),
                        stop=(kj == 4 * c + 3 and c > 0))
                if kj == 0 and c > 0:
                    # suffix adds commute with the accumulation: emit them
                    # up front so finalize()'s Ln can start the moment the
                    # last attnU matmul lands
                    for hl in range(2):
                        h = 2 * p + hl
                        for ql in range(4):
                            qi = 4 * c + ql
                            if qi < NQ - 1:
                                nc.tensor.matmul(
                                    aups[:, hl, ql * 128:(ql + 1) * 128],
                                    fs[:, h, qi, :], ones128,
                                    start=False, stop=False)
                for _ in range(per):
                    if fillers:
                        fillers.pop(0)()
        for f in fillers:
            f()

    def finalize(c):
        """FS adds, rowsum reciprocal, normalize into att for chunk c."""
        ch = slice(c * QCH, (c + 1) * QCH)
        for p in range(NPAIRS):
            aups = aups_tiles[(p, c)]
            if c == 0:  # fs not yet computed when chunk 0 was emitted
                for hl in range(2):
                    for ql in range(4):
                        nc.tensor.matmul(
                            aups[:, hl, ql * 128:(ql + 1) * 128],
                            fs[:, 2 * p + hl, 4 * c + ql, :], ones128,
                            start=False, stop=(ql == 3))
            lnr = pb_r.tile([128, 2 * QCH], F32, tag="lr", name=f"lnr{p}_{c}")
            nc.scalar.activation(lnr[64:65, :], aups[64:65, :, :], AF.Ln)
            rec = pb_r.tile([128, 2 * QCH], F32, tag="lr", name=f"rec{p}_{c}")
            nc.scalar.activation(rec[64:65, :], lnr[64:65, :], AF.Exp,
                                 scale=-1.0)
            rep = pb_r.tile([128, 2 * QCH], F32, tag="rep", name=f"rep{p}_{c}")
            r = p * NCH + c
            dma.dma_start(io["dscratch"][r:r + 1, :], rec[64:65, :])
            dma.dma_start(rep[0:64, :],
                          io["dscratch"][r:r + 1, :].broadcast_to([64, 2 * QCH]))
            for hl in range(2):
                nc.vector.tensor_mul(
                    att[hl * 64:(hl + 1) * 64, p, ch],
                    aups[0:64, hl, :],
                    rep[0:64, hl * QCH:(hl + 1) * QCH])

    def outproj_unit(st, dc):
        pso = pb_s.tile([128, 2, QCH], F32, tag="sps", name=f"pso{st}_{dc}")
        for p in range(NPAIRS):
            # K=128 contraction = both heads of the pair stacked
            nc.tensor.matmul(
                pso[:, 0, :],
                att[:, p, st * 128:(st + 1) * 128],
                wot[:, p, dc * 512:(dc + 1) * 512],
                start=(p == 0), stop=(p == NPAIRS - 1))
        ob = pb_e.tile([128, QCH], BF16, tag="ob", name=f"ob{st}_{dc}")
        nc.vector.tensor_copy(ob, pso[:, 0, :])
        dma.dma_start(io["out"][st * 128:(st + 1) * 128,
                                dc * 512:(dc + 1) * 512], ob)

    def outproj(c):
        for st in range(4 * c, 4 * c + 4):
            for dc in range(2):
                outproj_unit(st, dc)

    def op_fillers(c):
        return [lambda st=st, dc=dc: outproj_unit(st, dc)
                for st in range(4 * c, 4 * c + 4) for dc in range(2)]

    # Interleave projections with attention so ScalarE (exp) starts early.
    # finalize(c) needs ALL v2 blocks (its FS suffix sums reach to kj=15 —
    # a true data dependency of the reference), so finalizes wait until all
    # v-projections are emitted; aups double-buffering then bounds how many
    # chunk loops may run before the first finalize.
    qkproj(0)
    vproj(0)
    chunk_loop(0)
    vproj(1)
    vproj(2)
    vproj(3)
    folded_suffixes()
    qkproj(1)
    finalize(0)
    chunk_loop(1)
    qkproj(2)
    finalize(1)
    outproj(0)
    chunk_loop(2)
    qkproj(3)
    finalize(2)
    outproj(1)
    chunk_loop(3)
    finalize(3)
    outproj(2)
    outproj(3)

    pb_r.release()
    pb_e.release()
    pb_a.release()
    pb_s.release()
    persist.release()


_CACHED = None


def _patch_act_tables():
    """Make Exp and Ln resolve to the single combined table set so the
    per-chunk recip (Ln/Exp) doesn't thrash ACT_TABLE_LOADs against the
    softmax Exp calls. Set positions (= act_func_set_id) are preserved;
    only membership of Exp/Ln in other sets is hidden from the selector."""
    from concourse import hw_specs
    orig = hw_specs.get_activation_tables

    def patched(arch):
        t = dict(orig(arch))
        if "natural_log_exp_and_others" in t:
            for name in t:
                if name != "natural_log_exp_and_others":
                    t[name] = t[name] - {AF.Exp, AF.Ln}
        return t

    bacc.get_activation_tables = patched


def _build():
    global _CACHED
    if _CACHED is not None:
        return _CACHED
    _patch_act_tables()
    nc = bacc.Bacc("TRN2", target_bir_lowering=False, debug=False)
    io = {
        "QT": nc.dram_tensor("QT", [D, S], BF16, kind="ExternalInput").ap(),
        "KT": nc.dram_tensor("KT", [D, S], BF16, kind="ExternalInput").ap(),
        "VT": nc.dram_tensor("VT", [D, S], BF16, kind="ExternalInput").ap(),
        "WqT": nc.dram_tensor("WqT", [D, 256], BF16, kind="ExternalInput").ap(),
        "WkT": nc.dram_tensor("WkT", [D, 256], BF16, kind="ExternalInput").ap(),
        "WvT": nc.dram_tensor("WvT", [D, 256], BF16, kind="ExternalInput").ap(),
        "WoT": nc.dram_tensor("WoT", [256, D], BF16, kind="ExternalInput").ap(),
        "bq": nc.dram_tensor("bq", [1, 256], BF16, kind="ExternalInput").ap(),
        "bk": nc.dram_tensor("bk", [1, 256], BF16, kind="ExternalInput").ap(),
        "bv": nc.dram_tensor("bv", [1, 256], BF16, kind="ExternalInput").ap(),
        "out": nc.dram_tensor("out", [S, D], BF16, kind="ExternalOutput").ap(),
        "dscratch": nc.dram_tensor("dscratch", [NPAIRS * NCH, 2 * QCH], F32,
                                   kind="Internal").ap(),
    }
    with tile.TileContext(nc) as tc:
        _emit(tc, io)
    nc.compile()
    _CACHED = (nc, io)
    return _CACHED


def make_in_maps(Q, K, V, Wq, bq, Wk, bk, Wv, bv, Wo):
    """Build the 8 per-core input dicts (host-side sharding)."""
    Q = np.asarray(Q, np.float32)
    K = np.asarray(K, np.float32)
    V = np.asarray(V, np.float32)
    qt = [np.ascontiguousarray(Q[b].T).astype(NPBF16) for b in range(B)]
    kt = [np.ascontiguousarray(K[b].T).astype(NPBF16) for b in range(B)]
    vt = [np.ascontiguousarray(V[b].T).astype(NPBF16) for b in range(B)]
    in_maps = []
    for core in range(NCORES):
        b, g = divmod(core, 4)
        rows = slice(g * 256, (g + 1) * 256)
        in_maps.append({
            "QT": qt[b], "KT": kt[b], "VT": vt[b],
            "WqT": np.ascontiguousarray(np.asarray(Wq, np.float32)[rows].T).astype(NPBF16),
            "WkT": np.ascontiguousarray(np.asarray(Wk, np.float32)[rows].T).astype(NPBF16),
            "WvT": np.ascontiguousarray(np.asarray(Wv, np.float32)[rows].T).astype(NPBF16),
            "WoT": np.ascontiguousarray(np.asarray(Wo, np.float32)[:, rows].T).astype(NPBF16),
            "bq": np.asarray(bq, np.float32)[rows].reshape(1, 256).astype(NPBF16),
            "bk": np.asarray(bk, np.float32)[rows].reshape(1, 256).astype(NPBF16),
            "bv": np.asarray(bv, np.float32)[rows].reshape(1, 256).astype(NPBF16),
        })
    return in_maps


def kernel(Q, K, V, mask, Wq, bq, Wk, bk, Wv, bv, Wo, bo, _results_hook=None):
    nc, _io = _build()
    in_maps = make_in_maps(Q, K, V, Wq, bq, Wk, bk, Wv, bv, Wo)
    res = run_bass_kernel_spmd(nc, in_maps, core_ids=list(range(NCORES)))
    if _results_hook is not None:
        _results_hook(res)
    out = np.zeros((B, S, D), np.float32)
    for core in range(NCORES):
        out[core // 4] += np.asarray(res.results[core]["out"], np.float32)
    out += np.asarray(bo, np.float32)
    return out



# revision 10
# speedup vs baseline: 1.1981x; 1.1981x over previous
"""Self-contained Trainium2 Bass kernel for MultiHeadAttention (v3, bf16).

Problem: B=2, S=2048, D=1024, H=16, hd=64, with the reference's
masked_fill(mask==0, -1e-09) quirk: masked scores become ~0.0, so
exp(masked) == 1.0 in fp32 and every key position participates in the
softmax denominator. Fully-masked key blocks therefore contribute a
block-constant suffix sum of V rows (fs), added via cheap matmuls
instead of full score/attn matmuls.

All matmul numerics are bf16: fp8 anywhere in the value path costs
2-4.5% relative output error (softmax averaging shrinks the signal as
fast as the noise), beyond the 2e-2 gate.

v3 structural changes vs the original baseline:
  * Input DMA consolidated to ~16 large 3D-AP descriptors (was ~77),
    issued on sync/gpsimd so ScalarE keeps its cycles for exp.
  * v2 PSUM->SBUF copies batched: all 4 heads in one strided copy.
  * fs suffix adds batched across the 4 heads (15 TT ops, was 60).
  * Diagonal-mask affine_selects batched across the hl pair (32, was 64).
  * outproj emits one [128, 2, 512] PSUM tile per seq block: 4 matmuls,
    one DVE copy, one output DMA (was 2 copies + 2 DMAs).

Layouts (per core, all matmul operands at partition base 0):
  qt  [128, pair, S]   q^T, two heads stacked on partitions (d dims)
  ktz [128, head, S]   k^T zero-padded: even heads live on partitions
                       0-63 (64-127 zero), odd heads on 64-127 — the
                       scores matmul is then a plain K=128 matmul
                       against the pair-stacked qt.
  v2  [128, head, kj, 65]  V blocks with an appended ones column
                       (produces the softmax denominator for free).
  scores^T [sk, sq] in PSUM -> exp on ScalarE -> bf16 tiles ->
  attnU^T [65, sq] accumulated with V2 stationary, so no transposes
  are needed before the O-projection; rowsum = row 64.

Sharding: 8 cores = 2 batches x 4 head-groups (4 heads per core).
Each core computes a partial [S, D] output; the host sums the 4
partials per batch and adds bo.
"""

import numpy as np
import ml_dtypes

import concourse.bass as bass
import concourse.bacc as bacc
import concourse.tile as tile
import concourse.mybir as mybir
from concourse.bass_utils import run_bass_kernel_spmd

BF16 = mybir.dt.bfloat16
F32 = mybir.dt.float32
NPBF16 = ml_dtypes.bfloat16
AF = mybir.ActivationFunctionType

B = 2
S = 2048
D = 1024
H = 16
HD = 64
NCORES = 8
HPC = 4            # heads per core
NPAIRS = 2         # head pairs per core
NQ = S // 128      # 16 query/key blocks of 128
QCH = 512          # sq chunk width
NCH = S // QCH     # 4 chunks
KT = D // 128      # 8 contraction tiles for projections


def _emit(tc: tile.TileContext, io: dict):
    nc = tc.nc

    persist = tc.alloc_tile_pool(name="persist", bufs=1)

    # ---- constants ----
    ones128 = persist.tile([128, 128], BF16, name="ones128")
    nc.gpsimd.memset(ones128, 1.0)
    onesrow = persist.tile([1, 512], BF16, name="onesrow")
    nc.gpsimd.memset(onesrow, 1.0)

    # ---- persistent SBUF arrays ----
    qt = persist.tile([128, NPAIRS, S], BF16, name="qt")
    ktz = persist.tile([128, HPC, S], BF16, name="ktz")
    v2 = persist.tile([128, HPC, NQ, 65], BF16, name="v2")
    fs = persist.tile([128, HPC, NQ, 65], BF16, name="fs")
    att = persist.tile([128, NPAIRS, S], BF16, name="att")

    qts = persist.tile([128, KT, S], BF16, name="qts")
    kts = persist.tile([128, KT, S], BF16, name="kts")
    vts = persist.tile([128, KT, S], BF16, name="vts")
    wqt = persist.tile([128, KT, 256], BF16, name="wqt")
    wkt = persist.tile([128, KT, 256], BF16, name="wkt")
    wvt = persist.tile([128, KT, 256], BF16, name="wvt")
    wot = persist.tile([128, NPAIRS, D], BF16, name="wot")
    bq_sb = persist.tile([1, 256], BF16, name="bq_sb")
    bk_sb = persist.tile([1, 256], BF16, name="bk_sb")
    bv_sb = persist.tile([1, 256], BF16, name="bv_sb")

    # ---- input DMA: few large 3D-AP descriptors ----
    # gpsimd (SWDGE): small weight tensors, early; keeps ScalarE free
    nc.gpsimd.dma_start(bv_sb, io["bv"])
    nc.gpsimd.dma_start(bq_sb, io["bq"])
    nc.gpsimd.dma_start(bk_sb, io["bk"])
    nc.gpsimd.dma_start(wvt, io["WvT"].rearrange("(t p) m -> p t m", p=128))
    nc.gpsimd.dma_start(wqt, io["WqT"].rearrange("(t p) m -> p t m", p=128))
    nc.gpsimd.dma_start(wkt, io["WkT"].rearrange("(t p) m -> p t m", p=128))
    nc.gpsimd.dma_start(wot, io["WoT"].rearrange("(o p) m -> p o m", p=128))

    # sync (HWDGE): activations; chunk 0 first so projections start early
    def xt_src(t_io, c0, c1):
        return t_io.rearrange("(t p) s -> p t s", p=128)[:, :, c0:c1]

    dma = nc.sync
    dma.dma_start(qts[:, :, 0:QCH], xt_src(io["QT"], 0, QCH))
    dma.dma_start(kts[:, :, 0:QCH], xt_src(io["KT"], 0, QCH))
    dma.dma_start(vts[:, :, 0:QCH], xt_src(io["VT"], 0, QCH))
    dma.dma_start(qts[:, :, QCH:2 * QCH], xt_src(io["QT"], QCH, 2 * QCH))
    dma.dma_start(kts[:, :, QCH:2 * QCH], xt_src(io["KT"], QCH, 2 * QCH))
    dma.dma_start(vts[:, :, QCH:2 * QCH], xt_src(io["VT"], QCH, 2 * QCH))
    dma.dma_start(qts[:, :, 2 * QCH:], xt_src(io["QT"], 2 * QCH, S))
    dma.dma_start(kts[:, :, 2 * QCH:], xt_src(io["KT"], 2 * QCH, S))
    dma.dma_start(vts[:, :, 2 * QCH:], xt_src(io["VT"], 2 * QCH, S))

    for h in range(HPC):  # zero the unused half of each ktz head
        half = slice(64, 128) if h % 2 == 0 else slice(0, 64)
        nc.vector.memset(ktz[half, h, :], 0.0)
    nc.gpsimd.memset(v2[:, :, :, 64:65], 1.0)  # ones column

    pb_s = tc.alloc_tile_pool(name="pb_scores", bufs=2, space="PSUM")
    pb_a = tc.alloc_tile_pool(name="pb_attnu", bufs=2, space="PSUM")
    pb_e = tc.alloc_tile_pool(name="pb_exp", bufs=7)
    pb_r = tc.alloc_tile_pool(name="pb_recip", bufs=2)

    def vproj(c):
        """V projection for key blocks 4c..4c+3 -> v2 tiles."""
        for st in range(4 * c, 4 * c + 4):
            psv_t = pb_s.tile([128, 2, QCH], F32, tag="sps", name=f"ps_v{st}")
            ps_v = psv_t[:, 0, 0:256]
            for t in range(KT):
                nc.tensor.matmul(ps_v, vts[:, t, st * 128:(st + 1) * 128],
                                 wvt[:, t, :], start=(t == 0), stop=False)
            nc.tensor.matmul(ps_v, ones128[0:1, :], bv_sb,
                             start=False, stop=True)  # + bv rank-1
            # one strided copy: all 4 heads at once
            nc.vector.tensor_copy(v2[:, :, st, 0:64],
                                  ps_v.rearrange("p (h d) -> p h d", h=4))

    def qproj_unit(c, p):
        sq = slice(c * QCH, (c + 1) * QCH)
        psq_t = pb_s.tile([128, 2, QCH], F32, tag="sps", name=f"ps_q{p}_{c}")
        ps_q = psq_t[:, 0, :]
        for t in range(KT):
            nc.tensor.matmul(ps_q, wqt[:, t, p * 128:(p + 1) * 128],
                             qts[:, t, sq], start=(t == 0), stop=False)
        nc.tensor.matmul(ps_q, bq_sb[0:1, p * 128:(p + 1) * 128],
                         onesrow, start=False, stop=True)  # + bq rank-1
        nc.vector.tensor_copy(qt[:, p, sq], ps_q)

    def kproj_unit(c, p):
        sq = slice(c * QCH, (c + 1) * QCH)
        psk_t = pb_s.tile([128, 2, QCH], F32, tag="sps", name=f"ps_k{p}_{c}")
        ps_k = psk_t[:, 0, :]
        for t in range(KT):
            nc.tensor.matmul(ps_k, wkt[:, t, p * 128:(p + 1) * 128],
                             kts[:, t, sq], start=(t == 0), stop=False)
        nc.tensor.matmul(ps_k, bk_sb[0:1, p * 128:(p + 1) * 128],
                         onesrow, start=False, stop=True)  # + bk rank-1
        nc.vector.tensor_copy(ktz[0:64, 2 * p, sq], ps_k[0:64, :])
        nc.vector.tensor_copy(ktz[64:128, 2 * p + 1, sq], ps_k[64:128, :])

    def qkproj(c):
        for p in range(NPAIRS):
            qproj_unit(c, p)
            kproj_unit(c, p)

    def folded_suffixes():
        nc.vector.memset(fs[:, :, NQ - 1, :], 0.0)
        for q in range(NQ - 2, -1, -1):
            # all 4 heads in one strided TT add
            nc.vector.tensor_add(fs[:, :, q, :], fs[:, :, q + 1, :],
                                 v2[:, :, q + 1, :])

    aups_tiles = {}

    def chunk_loop(c):
        """scores -> exp -> attnU^T accumulation for chunk c, both pairs."""
        nkj = 4 * c + 4
        for p in range(NPAIRS):
            aups = pb_a.tile([65, 2, QCH], F32, tag="aups", name=f"aups{p}_{c}")
            aups_tiles[(p, c)] = aups
            for kj in range(nkj):
                c0 = max(kj - 4 * c, 0) * 128   # first valid col in chunk
                sps = pb_s.tile([128, 2, QCH], F32, tag="sps",
                                name=f"sps{p}_{c}_{kj}")
                for hl in range(2):
                    nc.tensor.matmul(
                        sps[:, hl, c0:QCH],
                        ktz[:, 2 * p + hl, kj * 128:(kj + 1) * 128],
                        qt[:, p, c * QCH + c0:(c + 1) * QCH],
                        start=True, stop=True)
                ext = pb_e.tile([128, 2, QCH], BF16, tag="ext",
                                name=f"ext{p}_{c}_{kj}")
                nc.scalar.activation(ext[:, :, c0:QCH], sps[:, :, c0:QCH],
                                     AF.Exp, scale=0.125)
                if kj >= 4 * c:  # diagonal block: masked exp entries -> 1.0
                    nc.gpsimd.affine_select(
                        out=ext[:, :, c0:c0 + 128],
                        in_=ext[:, :, c0:c0 + 128],
                        compare_op=mybir.AluOpType.is_ge,
                        fill=1.0, base=0,
                        pattern=[[0, 2], [1, 128]], channel_multiplier=-1)
                for hl in range(2):
                    # masked cols < c0 get their (block-constant)
                    # contribution from the early FS matmuls below
                    nc.tensor.matmul(
                        aups[:, hl, c0:QCH],
                        v2[:, 2 * p + hl, kj, :],
                        ext[:, hl, c0:QCH],
                        start=(kj == 0),
                        stop=(kj == nkj - 1 and c > 0))
                if kj == 0 and c > 0:
                    # suffix adds commute with the accumulation: emit them
                    # up front so finalize()'s Ln can start the moment the
                    # last attnU matmul lands
                    for hl in range(2):
                        h = 2 * p + hl
                        for ql in range(4):
                            qi = 4 * c + ql
                            if qi < NQ - 1:
                                nc.tensor.matmul(
                                    aups[:, hl, ql * 128:(ql + 1) * 128],
                                    fs[:, h, qi, :], ones128,
                                    start=False, stop=False)

    def finalize(c):
        """FS adds, rowsum reciprocal, normalize into att for chunk c."""
        ch = slice(c * QCH, (c + 1) * QCH)
        for p in range(NPAIRS):
            aups = aups_tiles[(p, c)]
            if c == 0:  # fs not yet computed when chunk 0 was emitted
                for hl in range(2):
                    for ql in range(4):
                        nc.tensor.matmul(
                            aups[:, hl, ql * 128:(ql + 1) * 128],
                            fs[:, 2 * p + hl, 4 * c + ql, :], ones128,
                            start=False, stop=(ql == 3))
            lnr = pb_r.tile([128, 2 * QCH], F32, tag="lr", name=f"lnr{p}_{c}")
            nc.scalar.activation(lnr[64:65, :], aups[64:65, :, :], AF.Ln)
            rec = pb_r.tile([128, 2 * QCH], F32, tag="lr", name=f"rec{p}_{c}")
            nc.scalar.activation(rec[64:65, :], lnr[64:65, :], AF.Exp,
                                 scale=-1.0)
            rep = pb_r.tile([128, 2 * QCH], F32, tag="rep", name=f"rep{p}_{c}")
            r = p * NCH + c
            dma.dma_start(io["dscratch"][r:r + 1, :], rec[64:65, :])
            dma.dma_start(rep[0:64, :],
                          io["dscratch"][r:r + 1, :].broadcast_to(
                              [64, 2 * QCH]))
            for hl in range(2):
                nc.vector.tensor_mul(
                    att[hl * 64:(hl + 1) * 64, p, ch],
                    aups[0:64, hl, :],
                    rep[0:64, hl * QCH:(hl + 1) * QCH])

    def outproj(c):
        for st in range(4 * c, 4 * c + 4):
            pso = pb_s.tile([128, 2, QCH], F32, tag="sps", name=f"pso{st}")
            for dc in range(2):
                for p in range(NPAIRS):
                    nc.tensor.matmul(
                        pso[:, dc, :],
                        att[:, p, st * 128:(st + 1) * 128],
                        wot[:, p, dc * 512:(dc + 1) * 512],
                        start=(p == 0), stop=(p == NPAIRS - 1))
            ob = pb_e.tile([128, 2, QCH], BF16, tag="ob", name=f"ob{st}")
            nc.vector.tensor_copy(ob, pso)
            dma.dma_start(
                io["out"][st * 128:(st + 1) * 128, :].rearrange(
                    "s (a m) -> s a m", a=2), ob)

    # Interleave projections with attention so ScalarE (exp) starts early.
    # finalize(c) needs ALL v2 blocks (its FS suffix sums reach to kj=15),
    # so finalizes wait until all v-projections are emitted.
    qkproj(0)
    vproj(0)
    chunk_loop(0)
    vproj(1)
    vproj(2)
    vproj(3)
    folded_suffixes()
    qkproj(1)
    finalize(0)
    chunk_loop(1)
    qkproj(2)
    finalize(1)
    outproj(0)
    chunk_loop(2)
    qkproj(3)
    finalize(2)
    outproj(1)
    chunk_loop(3)
    finalize(3)
    outproj(2)
    outproj(3)

    pb_r.release()
    pb_e.release()
    pb_a.release()
    pb_s.release()
    persist.release()


_CACHED = None


def _patch_act_tables():
    """Make Exp and Ln resolve to the single combined table set so the
    per-chunk recip (Ln/Exp) doesn't thrash ACT_TABLE_LOADs against the
    softmax Exp calls."""
    from concourse import hw_specs
    orig = hw_specs.get_activation_tables

    def patched(arch):
        t = dict(orig(arch))
        if "natural_log_exp_and_others" in t:
            for name in t:
                if name != "natural_log_exp_and_others":
                    t[name] = t[name] - {AF.Exp, AF.Ln}
        return t

    bacc.get_activation_tables = patched


def _build():
    global _CACHED
    if _CACHED is not None:
        return _CACHED
    _patch_act_tables()
    nc = bacc.Bacc("TRN2", target_bir_lowering=False, debug=False)
    io = {
        "QT": nc.dram_tensor("QT", [D, S], BF16, kind="ExternalInput").ap(),
        "KT": nc.dram_tensor("KT", [D, S], BF16, kind="ExternalInput").ap(),
        "VT": nc.dram_tensor("VT", [D, S], BF16, kind="ExternalInput").ap(),
        "WqT": nc.dram_tensor("WqT", [D, 256], BF16, kind="ExternalInput").ap(),
        "WkT": nc.dram_tensor("WkT", [D, 256], BF16, kind="ExternalInput").ap(),
        "WvT": nc.dram_tensor("WvT", [D, 256], BF16, kind="ExternalInput").ap(),
        "WoT": nc.dram_tensor("WoT", [256, D], BF16, kind="ExternalInput").ap(),
        "bq": nc.dram_tensor("bq", [1, 256], BF16, kind="ExternalInput").ap(),
        "bk": nc.dram_tensor("bk", [1, 256], BF16, kind="ExternalInput").ap(),
        "bv": nc.dram_tensor("bv", [1, 256], BF16, kind="ExternalInput").ap(),
        "out": nc.dram_tensor("out", [S, D], BF16, kind="ExternalOutput").ap(),
        "dscratch": nc.dram_tensor("dscratch", [NPAIRS * NCH, 2 * QCH], F32,
                                   kind="Internal").ap(),
    }
    with tile.TileContext(nc) as tc:
        _emit(tc, io)
    nc.compile()
    _CACHED = (nc, io)
    return _CACHED


def make_in_maps(Q, K, V, Wq, bq, Wk, bk, Wv, bv, Wo):
    """Build the 8 per-core input dicts (host-side sharding)."""
    Q = np.asarray(Q, np.float32)
    K = np.asarray(K, np.float32)
    V = np.asarray(V, np.float32)
    qt = [np.ascontiguousarray(Q[b].T).astype(NPBF16) for b in range(B)]
    kt = [np.ascontiguousarray(K[b].T).astype(NPBF16) for b in range(B)]
    vt = [np.ascontiguousarray(V[b].T).astype(NPBF16) for b in range(B)]
    in_maps = []
    for core in range(NCORES):
        b, g = divmod(core, 4)
        rows = slice(g * 256, (g + 1) * 256)
        in_maps.append({
            "QT": qt[b], "KT": kt[b], "VT": vt[b],
            "WqT": np.ascontiguousarray(
                np.asarray(Wq, np.float32)[rows].T).astype(NPBF16),
            "WkT": np.ascontiguousarray(
                np.asarray(Wk, np.float32)[rows].T).astype(NPBF16),
            "WvT": np.ascontiguousarray(
                np.asarray(Wv, np.float32)[rows].T).astype(NPBF16),
            "WoT": np.ascontiguousarray(
                np.asarray(Wo, np.float32)[:, rows].T).astype(NPBF16),
            "bq": np.asarray(bq, np.float32)[rows].reshape(1, 256
                                                           ).astype(NPBF16),
            "bk": np.asarray(bk, np.float32)[rows].reshape(1, 256
                                                           ).astype(NPBF16),
            "bv": np.asarray(bv, np.float32)[rows].reshape(1, 256
                                                           ).astype(NPBF16),
        })
    return in_maps


def kernel(Q, K, V, mask, Wq, bq, Wk, bk, Wv, bv, Wo, bo, _results_hook=None):
    nc, _io = _build()
    in_maps = make_in_maps(Q, K, V, Wq, bq, Wk, bk, Wv, bv, Wo)
    res = run_bass_kernel_spmd(nc, in_maps, core_ids=list(range(NCORES)))
    if _results_hook is not None:
        _results_hook(res)
    out = np.zeros((B, S, D), np.float32)
    for core in range(NCORES):
        out[core // 4] += np.asarray(res.results[core]["out"], np.float32)
    out += np.asarray(bo, np.float32)
    return out


# revision 26
# speedup vs baseline: 1.2310x; 1.0274x over previous
"""Self-contained Trainium2 Bass kernel for MultiHeadAttention (v3, bf16).

Problem: B=2, S=2048, D=1024, H=16, hd=64, with the reference's
masked_fill(mask==0, -1e-09) quirk: masked scores become ~0.0, so
exp(masked) == 1.0 in fp32 and every key position participates in the
softmax denominator. Fully-masked key blocks therefore contribute a
block-constant suffix sum of V rows (fs), added via cheap matmuls
instead of full score/attn matmuls.

All matmul numerics are bf16: fp8 anywhere in the value path costs
2-4.5% relative output error (softmax averaging shrinks the signal as
fast as the noise), beyond the 2e-2 gate.

v3 structural changes vs the original baseline:
  * Input DMA consolidated to ~16 large 3D-AP descriptors (was ~77),
    issued on sync/gpsimd so ScalarE keeps its cycles for exp.
  * v2 PSUM->SBUF copies batched: all 4 heads in one strided copy.
  * fs suffix adds batched across the 4 heads (15 TT ops, was 60).
  * Diagonal-mask affine_selects batched across the hl pair (32, was 64).
  * outproj emits one [128, 2, 512] PSUM tile per seq block: 4 matmuls,
    one DVE copy, one output DMA (was 2 copies + 2 DMAs).

Layouts (per core, all matmul operands at partition base 0):
  qt  [128, pair, S]   q^T, two heads stacked on partitions (d dims)
  ktz [128, head, S]   k^T zero-padded: even heads live on partitions
                       0-63 (64-127 zero), odd heads on 64-127 — the
                       scores matmul is then a plain K=128 matmul
                       against the pair-stacked qt.
  v2  [128, head, kj, 65]  V blocks with an appended ones column
                       (produces the softmax denominator for free).
  scores^T [sk, sq] in PSUM -> exp on ScalarE -> bf16 tiles ->
  attnU^T [65, sq] accumulated with V2 stationary, so no transposes
  are needed before the O-projection; rowsum = row 64.

Sharding: 8 cores = 2 batches x 4 head-groups (4 heads per core).
Each core computes a partial [S, D] output; the host sums the 4
partials per batch and adds bo.
"""

import numpy as np
import ml_dtypes

import concourse.bass as bass
import concourse.bacc as bacc
import concourse.tile as tile
import concourse.mybir as mybir
from concourse.bass_utils import run_bass_kernel_spmd

BF16 = mybir.dt.bfloat16
F32 = mybir.dt.float32
NPBF16 = ml_dtypes.bfloat16
AF = mybir.ActivationFunctionType

B = 2
S = 2048
D = 1024
H = 16
HD = 64
NCORES = 8
HPC = 4            # heads per core
NPAIRS = 2         # head pairs per core
NQ = S // 128      # 16 query/key blocks of 128
QCH = 512          # sq chunk width
NCH = S // QCH     # 4 chunks
KT = D // 128      # 8 contraction tiles for projections


def _emit(tc: tile.TileContext, io: dict):
    nc = tc.nc

    persist = tc.alloc_tile_pool(name="persist", bufs=1)

    # ---- constants ----
    ones128 = persist.tile([128, 128], BF16, name="ones128")
    nc.gpsimd.memset(ones128, 1.0)

    # ---- persistent SBUF arrays ----
    qt = persist.tile([128, NPAIRS, S], BF16, name="qt")
    ktz = persist.tile([128, HPC, S], BF16, name="ktz")
    v2 = persist.tile([128, HPC, NQ, 128], BF16, name="v2")
    fs = persist.tile([128, HPC, NQ, 128], BF16, name="fs")
    att = persist.tile([128, NPAIRS, S], BF16, name="att")

    qts = persist.tile([128, KT, S], BF16, name="qts")
    kts = persist.tile([128, KT, S], BF16, name="kts")
    vts = persist.tile([128, KT, S], BF16, name="vts")
    wqt = persist.tile([128, KT, 256], BF16, name="wqt")
    wkt = persist.tile([128, KT, 256], BF16, name="wkt")
    wvt = persist.tile([128, KT, 256], BF16, name="wvt")
    wot = persist.tile([128, NPAIRS, D], BF16, name="wot")
    # q/k biases as per-partition columns: bqc[:, p] = bq[p*128:(p+1)*128]
    bqc = persist.tile([128, NPAIRS], F32, name="bqc")
    bkc = persist.tile([128, NPAIRS], F32, name="bkc")

    # ---- input DMA: few large 3D-AP descriptors ----
    # gpsimd (SWDGE): small weight tensors, early; keeps ScalarE free
    nc.gpsimd.dma_start(wqt, io["WqT"].rearrange("(t p) m -> p t m", p=128))
    nc.gpsimd.dma_start(wkt, io["WkT"].rearrange("(t p) m -> p t m", p=128))
    nc.gpsimd.dma_start(wvt, io["WvT"].rearrange("(t p) m -> p t m", p=128))
    nc.gpsimd.dma_start(wot, io["WoT"].rearrange("(o p) m -> p o m", p=128))
    nc.scalar.dma_start(bqc, io["bqc"])
    nc.scalar.dma_start(bkc, io["bkc"])

    # sync (HWDGE): activations; chunk 0 first so projections start early
    def xt_src(t_io, c0, c1):
        return t_io.rearrange("(t p) s -> p t s", p=128)[:, :, c0:c1]

    dma = nc.sync
    # first chunk split by t-halves so the first projection matmuls can
    # start after ~0.5 MB; kts on the scalar HWDGE ring runs in parallel
    # with qts on the sync ring instead of queueing behind it.
    dma.dma_start(qts[:, 0:4, 0:QCH], xt_src(io["QT"], 0, QCH)[:, 0:4, :])
    dma.dma_start(qts[:, 4:8, 0:QCH], xt_src(io["QT"], 0, QCH)[:, 4:8, :])
    nc.scalar.dma_start(kts[:, 0:4, 0:QCH],
                        xt_src(io["KT"], 0, QCH)[:, 0:4, :])
    nc.scalar.dma_start(kts[:, 4:8, 0:QCH],
                        xt_src(io["KT"], 0, QCH)[:, 4:8, :])
    dma.dma_start(vts[:, :, 0:QCH], xt_src(io["VT"], 0, QCH))
    nc.scalar.dma_start(qts[:, :, QCH:2 * QCH], xt_src(io["QT"], QCH, 2 * QCH))
    dma.dma_start(kts[:, :, QCH:2 * QCH], xt_src(io["KT"], QCH, 2 * QCH))
    nc.scalar.dma_start(vts[:, :, QCH:2 * QCH], xt_src(io["VT"], QCH, 2 * QCH))
    dma.dma_start(qts[:, :, 2 * QCH:], xt_src(io["QT"], 2 * QCH, S))
    nc.scalar.dma_start(kts[:, :, 2 * QCH:], xt_src(io["KT"], 2 * QCH, S))
    dma.dma_start(vts[:, :, 2 * QCH:], xt_src(io["VT"], 2 * QCH, S))

    for h in range(HPC):  # zero the unused half of each ktz head
        half = slice(64, 128) if h % 2 == 0 else slice(0, 64)
        nc.vector.memset(ktz[half, h, :], 0.0)
    nc.gpsimd.memset(v2[:, :, :, 64:128], 1.0)  # 64 ones columns -> Z on rows 64-127

    pb_s = tc.alloc_tile_pool(name="pb_scores", bufs=2, space="PSUM")
    pb_a = tc.alloc_tile_pool(name="pb_attnu", bufs=2, space="PSUM")
    pb_e = tc.alloc_tile_pool(name="pb_exp", bufs=7)
    pb_o = tc.alloc_tile_pool(name="pb_ob", bufs=3)
    pb_r = tc.alloc_tile_pool(name="pb_recip", bufs=2)

    def vproj(c):
        """V projection for key blocks 4c..4c+3 -> v2 tiles."""
        for st in range(4 * c, 4 * c + 4):
            psv_t = pb_s.tile([128, 2, QCH], F32, tag="sps", name=f"ps_v{st}")
            ps_v = psv_t[:, 0, 0:256]
            for t in range(KT):
                nc.tensor.matmul(ps_v, vts[:, t, st * 128:(st + 1) * 128],
                                 wvt[:, t, :], start=(t == 0),
                                 stop=(t == KT - 1))
            # bv is NOT added here: it passes through the softmax average
            # exactly (weights sum to 1), so the host folds bv @ Wo.T into
            # the bo add instead.
            # one strided copy: all 4 heads at once
            nc.vector.tensor_copy(v2[:, :, st, 0:64],
                                  ps_v.rearrange("p (h d) -> p h d", h=4))

    def qproj_unit(c, p):
        sq = slice(c * QCH, (c + 1) * QCH)
        psq_t = pb_s.tile([128, 2, QCH], F32, tag="sps", name=f"ps_q{p}_{c}")
        ps_q = psq_t[:, 0, :]
        for t in range(KT):
            nc.tensor.matmul(ps_q, wqt[:, t, p * 128:(p + 1) * 128],
                             qts[:, t, sq], start=(t == 0),
                             stop=(t == KT - 1))
        # bias folded into the PSUM->SBUF copy (per-partition scalar add)
        nc.vector.tensor_scalar_add(qt[:, p, sq], ps_q, bqc[:, p:p + 1])

    def kproj_unit(c, p):
        sq = slice(c * QCH, (c + 1) * QCH)
        psk_t = pb_s.tile([128, 2, QCH], F32, tag="sps", name=f"ps_k{p}_{c}")
        ps_k = psk_t[:, 0, :]
        for t in range(KT):
            nc.tensor.matmul(ps_k, wkt[:, t, p * 128:(p + 1) * 128],
                             kts[:, t, sq], start=(t == 0),
                             stop=(t == KT - 1))
        nc.vector.tensor_scalar_add(ktz[0:64, 2 * p, sq], ps_k[0:64, :],
                                    bkc[0:64, p:p + 1])
        nc.vector.tensor_scalar_add(ktz[64:128, 2 * p + 1, sq],
                                    ps_k[64:128, :], bkc[64:128, p:p + 1])

    def qkproj(c):
        for p in range(NPAIRS):
            qproj_unit(c, p)
            kproj_unit(c, p)

    def folded_suffixes():
        nc.vector.memset(fs[:, :, NQ - 1, :], 0.0)
        for q in range(NQ - 2, -1, -1):
            # all 4 heads in one strided TT add
            nc.vector.tensor_add(fs[:, :, q, :], fs[:, :, q + 1, :],
                                 v2[:, :, q + 1, :])

    aups_tiles = {}

    def chunk_loop(c):
        """scores -> exp -> attnU^T accumulation for chunk c, both pairs."""
        nkj = 4 * c + 4
        for p in range(NPAIRS):
            aups = pb_a.tile([128, 2, QCH], F32, tag="aups", name=f"aups{p}_{c}")
            aups_tiles[(p, c)] = aups
            for kj in range(nkj):
                c0 = max(kj - 4 * c, 0) * 128   # first valid col in chunk
                sps = pb_s.tile([128, 2, QCH], F32, tag="sps",
                                name=f"sps{p}_{c}_{kj}")
                for hl in range(2):
                    nc.tensor.matmul(
                        sps[:, hl, c0:QCH],
                        ktz[:, 2 * p + hl, kj * 128:(kj + 1) * 128],
                        qt[:, p, c * QCH + c0:(c + 1) * QCH],
                        start=True, stop=True)
                ext = pb_e.tile([128, 2, QCH], BF16, tag="ext",
                                name=f"ext{p}_{c}_{kj}")
                nc.scalar.activation(ext[:, :, c0:QCH], sps[:, :, c0:QCH],
                                     AF.Exp, scale=0.125)
                if kj >= 4 * c:  # diagonal block: masked exp entries -> 1.0
                    nc.gpsimd.affine_select(
                        out=ext[:, :, c0:c0 + 128],
                        in_=ext[:, :, c0:c0 + 128],
                        compare_op=mybir.AluOpType.is_ge,
                        fill=1.0, base=0,
                        pattern=[[0, 2], [1, 128]], channel_multiplier=-1)
                for hl in range(2):
                    # masked cols < c0 get their (block-constant)
                    # contribution from the early FS matmuls below
                    nc.tensor.matmul(
                        aups[:, hl, c0:QCH],
                        v2[:, 2 * p + hl, kj, :],
                        ext[:, hl, c0:QCH],
                        start=(kj == 0),
                        stop=(kj == nkj - 1 and c > 0))
                if kj == 0 and c > 0:
                    # suffix adds commute with the accumulation: emit them
                    # up front so finalize()'s Ln can start the moment the
                    # last attnU matmul lands
                    for hl in range(2):
                        h = 2 * p + hl
                        for ql in range(4):
                            qi = 4 * c + ql
                            if qi < NQ - 1:
                                nc.tensor.matmul(
                                    aups[:, hl, ql * 128:(ql + 1) * 128],
                                    fs[:, h, qi, :], ones128,
                                    start=False, stop=False)

    def finalize(c):
        """FS adds, rowsum reciprocal, normalize into att for chunk c."""
        ch = slice(c * QCH, (c + 1) * QCH)
        for p in range(NPAIRS):
            aups = aups_tiles[(p, c)]
            if c == 0:  # fs not yet computed when chunk 0 was emitted
                for hl in range(2):
                    for ql in range(4):
                        nc.tensor.matmul(
                            aups[:, hl, ql * 128:(ql + 1) * 128],
                            fs[:, 2 * p + hl, 4 * c + ql, :], ones128,
                            start=False, stop=(ql == 3))
            lnr = pb_r.tile([128, 2 * QCH], F32, tag="lr", name=f"lnr{p}_{c}")
            nc.scalar.activation(lnr[64:128, :], aups[64:128, :, :], AF.Ln)
            nc.scalar.activation(lnr[64:128, :], lnr[64:128, :], AF.Exp,
                                 scale=-1.0)  # in-place: lnr becomes 1/Z
            for hl in range(2):
                nc.vector.tensor_mul(
                    att[hl * 64:(hl + 1) * 64, p, ch],
                    aups[0:64, hl, :],
                    lnr[64:128, hl * QCH:(hl + 1) * QCH])

    def outproj(c):
        for st in range(4 * c, 4 * c + 4):
            pso = pb_s.tile([128, 2, QCH], F32, tag="sps", name=f"pso{st}")
            for dc in range(2):
                for p in range(NPAIRS):
                    nc.tensor.matmul(
                        pso[:, dc, :],
                        att[:, p, st * 128:(st + 1) * 128],
                        wot[:, p, dc * 512:(dc + 1) * 512],
                        start=(p == 0), stop=(p == NPAIRS - 1))
            ob = pb_o.tile([128, 2, QCH], BF16, tag="ob", name=f"ob{st}")
            nc.vector.tensor_copy(ob, pso)
            dma.dma_start(
                io["out"][st * 128:(st + 1) * 128, :].rearrange(
                    "s (a m) -> s a m", a=2), ob)

    # Interleave projections with attention so ScalarE (exp) starts early.
    # finalize(c) needs ALL v2 blocks (its FS suffix sums reach to kj=15),
    # so finalizes wait until all v-projections are emitted.
    qkproj(0)
    vproj(0)
    chunk_loop(0)
    vproj(1)
    vproj(2)
    vproj(3)
    folded_suffixes()
    qkproj(1)
    finalize(0)
    chunk_loop(1)
    qkproj(2)
    finalize(1)
    outproj(0)
    chunk_loop(2)
    qkproj(3)
    finalize(2)
    outproj(1)
    outproj(2)
    chunk_loop(3)
    finalize(3)
    outproj(3)

    pb_r.release()
    pb_o.release()
    pb_e.release()
    pb_a.release()
    pb_s.release()
    persist.release()


_CACHED = None


def _patch_act_tables():
    """Make Exp and Ln resolve to the single combined table set so the
    per-chunk recip (Ln/Exp) doesn't thrash ACT_TABLE_LOADs against the
    softmax Exp calls."""
    from concourse import hw_specs
    orig = hw_specs.get_activation_tables

    def patched(arch):
        t = dict(orig(arch))
        if "natural_log_exp_and_others" in t:
            for name in t:
                if name != "natural_log_exp_and_others":
                    t[name] = t[name] - {AF.Exp, AF.Ln}
        return t

    bacc.get_activation_tables = patched


def _build():
    global _CACHED
    if _CACHED is not None:
        return _CACHED
    _patch_act_tables()
    nc = bacc.Bacc("TRN2", target_bir_lowering=False, debug=False)
    io = {
        "QT": nc.dram_tensor("QT", [D, S], BF16, kind="ExternalInput").ap(),
        "KT": nc.dram_tensor("KT", [D, S], BF16, kind="ExternalInput").ap(),
        "VT": nc.dram_tensor("VT", [D, S], BF16, kind="ExternalInput").ap(),
        "WqT": nc.dram_tensor("WqT", [D, 256], BF16, kind="ExternalInput").ap(),
        "WkT": nc.dram_tensor("WkT", [D, 256], BF16, kind="ExternalInput").ap(),
        "WvT": nc.dram_tensor("WvT", [D, 256], BF16, kind="ExternalInput").ap(),
        "WoT": nc.dram_tensor("WoT", [256, D], BF16, kind="ExternalInput").ap(),
        "bqc": nc.dram_tensor("bqc", [128, NPAIRS], F32,
                              kind="ExternalInput").ap(),
        "bkc": nc.dram_tensor("bkc", [128, NPAIRS], F32,
                              kind="ExternalInput").ap(),
        "out": nc.dram_tensor("out", [S, D], BF16, kind="ExternalOutput").ap(),
    }
    with tile.TileContext(nc) as tc:
        _emit(tc, io)
    nc.compile()
    _CACHED = (nc, io)
    return _CACHED


def make_in_maps(Q, K, V, Wq, bq, Wk, bk, Wv, bv, Wo):
    """Build the 8 per-core input dicts (host-side sharding)."""
    Q = np.asarray(Q, np.float32)
    K = np.asarray(K, np.float32)
    V = np.asarray(V, np.float32)
    qt = [np.ascontiguousarray(Q[b].T).astype(NPBF16) for b in range(B)]
    kt = [np.ascontiguousarray(K[b].T).astype(NPBF16) for b in range(B)]
    vt = [np.ascontiguousarray(V[b].T).astype(NPBF16) for b in range(B)]
    in_maps = []
    for core in range(NCORES):
        b, g = divmod(core, 4)
        rows = slice(g * 256, (g + 1) * 256)
        in_maps.append({
            "QT": qt[b], "KT": kt[b], "VT": vt[b],
            "WqT": np.ascontiguousarray(
                np.asarray(Wq, np.float32)[rows].T).astype(NPBF16),
            "WkT": np.ascontiguousarray(
                np.asarray(Wk, np.float32)[rows].T).astype(NPBF16),
            "WvT": np.ascontiguousarray(
                np.asarray(Wv, np.float32)[rows].T).astype(NPBF16),
            "WoT": np.ascontiguousarray(
                np.asarray(Wo, np.float32)[:, rows].T).astype(NPBF16),
            "bqc": np.ascontiguousarray(
                np.asarray(bq, np.float32)[rows].reshape(2, 128).T),
            "bkc": np.ascontiguousarray(
                np.asarray(bk, np.float32)[rows].reshape(2, 128).T),
        })
    return in_maps


def kernel(Q, K, V, mask, Wq, bq, Wk, bk, Wv, bv, Wo, bo, _results_hook=None):
    nc, _io = _build()
    in_maps = make_in_maps(Q, K, V, Wq, bq, Wk, bk, Wv, bv, Wo)
    res = run_bass_kernel_spmd(nc, in_maps, core_ids=list(range(NCORES)))
    if _results_hook is not None:
        _results_hook(res)
    out = np.zeros((B, S, D), np.float32)
    for core in range(NCORES):
        out[core // 4] += np.asarray(res.results[core]["out"], np.float32)
    # bv passes through the softmax average exactly; its output-space
    # contribution is the constant row bv @ Wo.T, folded in here.
    out += np.asarray(bo, np.float32) + (
        np.asarray(bv, np.float32) @ np.asarray(Wo, np.float32).T)
    return out


# revision 27
# speedup vs baseline: 1.3129x; 1.0665x over previous
"""Self-contained Trainium2 Bass kernel for MultiHeadAttention (v3, bf16).

Problem: B=2, S=2048, D=1024, H=16, hd=64, with the reference's
masked_fill(mask==0, -1e-09) quirk: masked scores become ~0.0, so
exp(masked) == 1.0 in fp32 and every key position participates in the
softmax denominator. Fully-masked key blocks therefore contribute a
block-constant suffix sum of V rows (fs), added via cheap matmuls
instead of full score/attn matmuls.

All matmul numerics are bf16: fp8 anywhere in the value path costs
2-4.5% relative output error (softmax averaging shrinks the signal as
fast as the noise), beyond the 2e-2 gate.

v3 structural changes vs the original baseline:
  * Input DMA consolidated to ~16 large 3D-AP descriptors (was ~77),
    issued on sync/gpsimd so ScalarE keeps its cycles for exp.
  * v2 PSUM->SBUF copies batched: all 4 heads in one strided copy.
  * fs suffix adds batched across the 4 heads (15 TT ops, was 60).
  * Diagonal-mask affine_selects batched across the hl pair (32, was 64).
  * outproj emits one [128, 2, 512] PSUM tile per seq block: 4 matmuls,
    one DVE copy, one output DMA (was 2 copies + 2 DMAs).

Layouts (per core, all matmul operands at partition base 0):
  qt  [128, pair, S]   q^T, two heads stacked on partitions (d dims)
  ktz [128, head, S]   k^T zero-padded: even heads live on partitions
                       0-63 (64-127 zero), odd heads on 64-127 — the
                       scores matmul is then a plain K=128 matmul
                       against the pair-stacked qt.
  v2  [128, head, kj, 65]  V blocks with an appended ones column
                       (produces the softmax denominator for free).
  scores^T [sk, sq] in PSUM -> exp on ScalarE -> bf16 tiles ->
  attnU^T [65, sq] accumulated with V2 stationary, so no transposes
  are needed before the O-projection; rowsum = row 64.

Sharding: 8 cores = 2 batches x 4 head-groups (4 heads per core).
Each core computes a partial [S, D] output; the host sums the 4
partials per batch and adds bo.
"""

import numpy as np
import ml_dtypes

import concourse.bass as bass
import concourse.bacc as bacc
import concourse.tile as tile
import concourse.mybir as mybir
from concourse.bass_utils import run_bass_kernel_spmd

BF16 = mybir.dt.bfloat16
F32 = mybir.dt.float32
NPBF16 = ml_dtypes.bfloat16
AF = mybir.ActivationFunctionType

B = 2
S = 2048
D = 1024
H = 16
HD = 64
NCORES = 8
HPC = 4            # heads per core
NPAIRS = 2         # head pairs per core
NQ = S // 128      # 16 query/key blocks of 128
QCH = 512          # sq chunk width
NCH = S // QCH     # 4 chunks
KT = D // 128      # 8 contraction tiles for projections


def _emit(tc: tile.TileContext, io: dict):
    nc = tc.nc

    persist = tc.alloc_tile_pool(name="persist", bufs=1)

    # ---- constants ----
    ones128 = persist.tile([128, 128], BF16, name="ones128")
    nc.gpsimd.memset(ones128, 1.0)

    # ---- persistent SBUF arrays ----
    qt = persist.tile([128, NPAIRS, S], BF16, name="qt")
    ktz = persist.tile([128, HPC, S], BF16, name="ktz")
    v2 = persist.tile([128, HPC, NQ, 128], BF16, name="v2")
    fs = persist.tile([128, HPC, NQ, 128], BF16, name="fs")
    att = persist.tile([128, NPAIRS, S], BF16, name="att")

    qts = persist.tile([128, KT, S], BF16, name="qts")
    kts = persist.tile([128, KT, S], BF16, name="kts")
    vts = persist.tile([128, KT, S], BF16, name="vts")
    wqt = persist.tile([128, KT, 256], BF16, name="wqt")
    wkt = persist.tile([128, KT, 256], BF16, name="wkt")
    wvt = persist.tile([128, KT, 256], BF16, name="wvt")
    wot = persist.tile([128, NPAIRS, D], BF16, name="wot")
    # q/k biases as per-partition columns: bqc[:, p] = bq[p*128:(p+1)*128]
    bqc = persist.tile([128, NPAIRS], F32, name="bqc")
    bkc = persist.tile([128, NPAIRS], F32, name="bkc")

    # ---- input DMA: few large 3D-AP descriptors ----
    # gpsimd (SWDGE): small weight tensors, early; keeps ScalarE free
    nc.gpsimd.dma_start(wqt, io["WqT"].rearrange("(t p) m -> p t m", p=128))
    nc.gpsimd.dma_start(wkt, io["WkT"].rearrange("(t p) m -> p t m", p=128))
    nc.gpsimd.dma_start(wvt, io["WvT"].rearrange("(t p) m -> p t m", p=128))
    nc.gpsimd.dma_start(wot, io["WoT"].rearrange("(o p) m -> p o m", p=128))
    nc.scalar.dma_start(bqc, io["bqc"])
    nc.scalar.dma_start(bkc, io["bkc"])

    # sync (HWDGE): activations; chunk 0 first so projections start early
    def xt_src(t_io, c0, c1):
        return t_io.rearrange("(t p) s -> p t s", p=128)[:, :, c0:c1]

    dma = nc.sync
    # All big transfers on the sync HWDGE ring (issuing a large DMA can
    # block the issuing engine for ~10us -> never on ScalarE).  First
    # chunk split by t-halves so the first projection matmuls start
    # after ~0.5 MB; vts prioritized because the fs suffix chain
    # (vproj -> v2 -> fs) gates the first finalize.
    dma.dma_start(qts[:, 0:4, 0:QCH], xt_src(io["QT"], 0, QCH)[:, 0:4, :])
    dma.dma_start(qts[:, 4:8, 0:QCH], xt_src(io["QT"], 0, QCH)[:, 4:8, :])
    dma.dma_start(kts[:, 0:4, 0:QCH], xt_src(io["KT"], 0, QCH)[:, 0:4, :])
    dma.dma_start(kts[:, 4:8, 0:QCH], xt_src(io["KT"], 0, QCH)[:, 4:8, :])
    dma.dma_start(vts[:, :, 0:QCH], xt_src(io["VT"], 0, QCH))
    dma.dma_start(vts[:, :, QCH:2 * QCH], xt_src(io["VT"], QCH, 2 * QCH))
    dma.dma_start(qts[:, :, QCH:2 * QCH], xt_src(io["QT"], QCH, 2 * QCH))
    dma.dma_start(kts[:, :, QCH:2 * QCH], xt_src(io["KT"], QCH, 2 * QCH))
    dma.dma_start(vts[:, :, 2 * QCH:], xt_src(io["VT"], 2 * QCH, S))
    dma.dma_start(qts[:, :, 2 * QCH:], xt_src(io["QT"], 2 * QCH, S))
    dma.dma_start(kts[:, :, 2 * QCH:], xt_src(io["KT"], 2 * QCH, S))

    for h in range(HPC):  # zero the unused half of each ktz head
        half = slice(64, 128) if h % 2 == 0 else slice(0, 64)
        nc.vector.memset(ktz[half, h, :], 0.0)
    nc.gpsimd.memset(v2[:, :, :, 64:128], 1.0)  # 64 ones columns -> Z on rows 64-127

    pb_s = tc.alloc_tile_pool(name="pb_scores", bufs=2, space="PSUM")
    pb_a = tc.alloc_tile_pool(name="pb_attnu", bufs=2, space="PSUM")
    pb_e = tc.alloc_tile_pool(name="pb_exp", bufs=7)
    pb_o = tc.alloc_tile_pool(name="pb_ob", bufs=3)
    pb_r = tc.alloc_tile_pool(name="pb_recip", bufs=2)

    def vproj(c):
        """V projection for key blocks 4c..4c+3 -> v2 tiles."""
        for st in range(4 * c, 4 * c + 4):
            psv_t = pb_s.tile([128, 2, QCH], F32, tag="sps", name=f"ps_v{st}")
            ps_v = psv_t[:, 0, 0:256]
            for t in range(KT):
                nc.tensor.matmul(ps_v, vts[:, t, st * 128:(st + 1) * 128],
                                 wvt[:, t, :], start=(t == 0),
                                 stop=(t == KT - 1))
            # bv is NOT added here: it passes through the softmax average
            # exactly (weights sum to 1), so the host folds bv @ Wo.T into
            # the bo add instead.
            # one strided copy: all 4 heads at once
            nc.vector.tensor_copy(v2[:, :, st, 0:64],
                                  ps_v.rearrange("p (h d) -> p h d", h=4))

    def qproj_unit(c, p):
        sq = slice(c * QCH, (c + 1) * QCH)
        psq_t = pb_s.tile([128, 2, QCH], F32, tag="sps", name=f"ps_q{p}_{c}")
        ps_q = psq_t[:, 0, :]
        for t in range(KT):
            nc.tensor.matmul(ps_q, wqt[:, t, p * 128:(p + 1) * 128],
                             qts[:, t, sq], start=(t == 0),
                             stop=(t == KT - 1))
        # bias folded into the PSUM->SBUF copy (per-partition scalar add)
        nc.vector.tensor_scalar_add(qt[:, p, sq], ps_q, bqc[:, p:p + 1])

    def kproj_unit(c, p):
        sq = slice(c * QCH, (c + 1) * QCH)
        psk_t = pb_s.tile([128, 2, QCH], F32, tag="sps", name=f"ps_k{p}_{c}")
        ps_k = psk_t[:, 0, :]
        for t in range(KT):
            nc.tensor.matmul(ps_k, wkt[:, t, p * 128:(p + 1) * 128],
                             kts[:, t, sq], start=(t == 0),
                             stop=(t == KT - 1))
        nc.vector.tensor_scalar_add(ktz[0:64, 2 * p, sq], ps_k[0:64, :],
                                    bkc[0:64, p:p + 1])
        nc.vector.tensor_scalar_add(ktz[64:128, 2 * p + 1, sq],
                                    ps_k[64:128, :], bkc[64:128, p:p + 1])

    def qkproj(c):
        for p in range(NPAIRS):
            qproj_unit(c, p)
            kproj_unit(c, p)

    def folded_suffixes():
        nc.vector.memset(fs[:, :, NQ - 1, :], 0.0)
        for q in range(NQ - 2, -1, -1):
            # all 4 heads in one strided TT add
            nc.vector.tensor_add(fs[:, :, q, :], fs[:, :, q + 1, :],
                                 v2[:, :, q + 1, :])

    aups_tiles = {}

    def chunk_loop(c):
        """scores -> exp -> attnU^T accumulation for chunk c, both pairs.

        Software-pipelined one kj deep: the PE queue is strict FIFO, so
        scores(kj+1) must be emitted BEFORE attnU(kj) or every kj pays
        the full exp latency serially."""
        nkj = 4 * c + 4
        exts = {}

        def scores_unit(p, kj):
            c0 = max(kj - 4 * c, 0) * 128   # first valid col in chunk
            sps = pb_s.tile([128, 2, QCH], F32, tag="sps",
                            name=f"sps{p}_{c}_{kj}")
            for hl in range(2):
                nc.tensor.matmul(
                    sps[:, hl, c0:QCH],
                    ktz[:, 2 * p + hl, kj * 128:(kj + 1) * 128],
                    qt[:, p, c * QCH + c0:(c + 1) * QCH],
                    start=True, stop=True)
            ext = pb_e.tile([128, 2, QCH], BF16, tag="ext",
                            name=f"ext{p}_{c}_{kj}")
            nc.scalar.activation(ext[:, :, c0:QCH], sps[:, :, c0:QCH],
                                 AF.Exp, scale=0.125)
            if kj >= 4 * c:  # diagonal block: masked exp entries -> 1.0
                nc.gpsimd.affine_select(
                    out=ext[:, :, c0:c0 + 128],
                    in_=ext[:, :, c0:c0 + 128],
                    compare_op=mybir.AluOpType.is_ge,
                    fill=1.0, base=0,
                    pattern=[[0, 2], [1, 128]], channel_multiplier=-1)
            exts[(p, kj)] = ext

        def attnu_unit(p, kj, aups):
            c0 = max(kj - 4 * c, 0) * 128
            ext = exts.pop((p, kj))
            for hl in range(2):
                # masked cols < c0 get their (block-constant)
                # contribution from the FS matmuls
                nc.tensor.matmul(
                    aups[:, hl, c0:QCH],
                    v2[:, 2 * p + hl, kj, :],
                    ext[:, hl, c0:QCH],
                    start=(kj == 0),
                    stop=(kj == nkj - 1 and c > 0))

        for p in range(NPAIRS):
            aups = pb_a.tile([128, 2, QCH], F32, tag="aups", name=f"aups{p}_{c}")
            aups_tiles[(p, c)] = aups
            scores_unit(p, 0)
            for kj in range(nkj):
                if kj + 1 < nkj:
                    scores_unit(p, kj + 1)
                attnu_unit(p, kj, aups)
                if c > 0 and 1 <= kj <= 4:
                    # fs suffix adds, spread one per kj, in the order the
                    # fs chain produces them (high qi first); they commute
                    # with the accumulation
                    ql = 4 - kj
                    qi = 4 * c + ql
                    if qi < NQ - 1:
                        for hl in range(2):
                            nc.tensor.matmul(
                                aups[:, hl, ql * 128:(ql + 1) * 128],
                                fs[:, 2 * p + hl, qi, :], ones128,
                                start=False, stop=False)

    def finalize(c):
        """FS adds, rowsum reciprocal, normalize into att for chunk c."""
        ch = slice(c * QCH, (c + 1) * QCH)
        for p in range(NPAIRS):
            aups = aups_tiles[(p, c)]
            if c == 0:  # fs not yet computed when chunk 0 was emitted
                for hl in range(2):
                    for ql in range(4):
                        nc.tensor.matmul(
                            aups[:, hl, ql * 128:(ql + 1) * 128],
                            fs[:, 2 * p + hl, 4 * c + ql, :], ones128,
                            start=False, stop=(ql == 3))
            lnr = pb_r.tile([128, 2 * QCH], F32, tag="lr", name=f"lnr{p}_{c}")
            nc.scalar.activation(lnr[64:128, :], aups[64:128, :, :], AF.Ln)
            nc.scalar.activation(lnr[64:128, :], lnr[64:128, :], AF.Exp,
                                 scale=-1.0)  # in-place: lnr becomes 1/Z
            for hl in range(2):
                nc.vector.tensor_mul(
                    att[hl * 64:(hl + 1) * 64, p, ch],
                    aups[0:64, hl, :],
                    lnr[64:128, hl * QCH:(hl + 1) * QCH])

    def outproj(c):
        for st in range(4 * c, 4 * c + 4):
            pso = pb_s.tile([128, 2, QCH], F32, tag="sps", name=f"pso{st}")
            for dc in range(2):
                for p in range(NPAIRS):
                    nc.tensor.matmul(
                        pso[:, dc, :],
                        att[:, p, st * 128:(st + 1) * 128],
                        wot[:, p, dc * 512:(dc + 1) * 512],
                        start=(p == 0), stop=(p == NPAIRS - 1))
            ob = pb_o.tile([128, 2, QCH], BF16, tag="ob", name=f"ob{st}")
            nc.vector.tensor_copy(ob, pso)
            dma.dma_start(
                io["out"][st * 128:(st + 1) * 128, :].rearrange(
                    "s (a m) -> s a m", a=2), ob)

    # Interleave projections with attention so ScalarE (exp) starts early.
    # finalize(c) needs ALL v2 blocks (its FS suffix sums reach to kj=15),
    # so finalizes wait until all v-projections are emitted.
    qkproj(0)
    vproj(0)
    vproj(1)
    chunk_loop(0)
    vproj(2)
    vproj(3)
    folded_suffixes()
    qkproj(1)
    finalize(0)
    chunk_loop(1)
    qkproj(2)
    finalize(1)
    outproj(0)
    chunk_loop(2)
    qkproj(3)
    finalize(2)
    outproj(1)
    outproj(2)
    chunk_loop(3)
    finalize(3)
    outproj(3)

    pb_r.release()
    pb_o.release()
    pb_e.release()
    pb_a.release()
    pb_s.release()
    persist.release()


_CACHED = None


def _patch_act_tables():
    """Make Exp and Ln resolve to the single combined table set so the
    per-chunk recip (Ln/Exp) doesn't thrash ACT_TABLE_LOADs against the
    softmax Exp calls."""
    from concourse import hw_specs
    orig = hw_specs.get_activation_tables

    def patched(arch):
        t = dict(orig(arch))
        if "natural_log_exp_and_others" in t:
            for name in t:
                if name != "natural_log_exp_and_others":
                    t[name] = t[name] - {AF.Exp, AF.Ln}
        return t

    bacc.get_activation_tables = patched


def _build():
    global _CACHED
    if _CACHED is not None:
        return _CACHED
    _patch_act_tables()
    nc = bacc.Bacc("TRN2", target_bir_lowering=False, debug=False)
    io = {
        "QT": nc.dram_tensor("QT", [D, S], BF16, kind="ExternalInput").ap(),
        "KT": nc.dram_tensor("KT", [D, S], BF16, kind="ExternalInput").ap(),
        "VT": nc.dram_tensor("VT", [D, S], BF16, kind="ExternalInput").ap(),
        "WqT": nc.dram_tensor("WqT", [D, 256], BF16, kind="ExternalInput").ap(),
        "WkT": nc.dram_tensor("WkT", [D, 256], BF16, kind="ExternalInput").ap(),
        "WvT": nc.dram_tensor("WvT", [D, 256], BF16, kind="ExternalInput").ap(),
        "WoT": nc.dram_tensor("WoT", [256, D], BF16, kind="ExternalInput").ap(),
        "bqc": nc.dram_tensor("bqc", [128, NPAIRS], F32,
                              kind="ExternalInput").ap(),
        "bkc": nc.dram_tensor("bkc", [128, NPAIRS], F32,
                              kind="ExternalInput").ap(),
        "out": nc.dram_tensor("out", [S, D], BF16, kind="ExternalOutput").ap(),
    }
    with tile.TileContext(nc) as tc:
        _emit(tc, io)
    nc.compile()
    _CACHED = (nc, io)
    return _CACHED


def make_in_maps(Q, K, V, Wq, bq, Wk, bk, Wv, bv, Wo):
    """Build the 8 per-core input dicts (host-side sharding)."""
    Q = np.asarray(Q, np.float32)
    K = np.asarray(K, np.float32)
    V = np.asarray(V, np.float32)
    qt = [np.ascontiguousarray(Q[b].T).astype(NPBF16) for b in range(B)]
    kt = [np.ascontiguousarray(K[b].T).astype(NPBF16) for b in range(B)]
    vt = [np.ascontiguousarray(V[b].T).astype(NPBF16) for b in range(B)]
    in_maps = []
    for core in range(NCORES):
        b, g = divmod(core, 4)
        rows = slice(g * 256, (g + 1) * 256)
        in_maps.append({
            "QT": qt[b], "KT": kt[b], "VT": vt[b],
            "WqT": np.ascontiguousarray(
                np.asarray(Wq, np.float32)[rows].T).astype(NPBF16),
            "WkT": np.ascontiguousarray(
                np.asarray(Wk, np.float32)[rows].T).astype(NPBF16),
            "WvT": np.ascontiguousarray(
                np.asarray(Wv, np.float32)[rows].T).astype(NPBF16),
            "WoT": np.ascontiguousarray(
                np.asarray(Wo, np.float32)[:, rows].T).astype(NPBF16),
            "bqc": np.ascontiguousarray(
                np.asarray(bq, np.float32)[rows].reshape(2, 128).T),
            "bkc": np.ascontiguousarray(
                np.asarray(bk, np.float32)[rows].reshape(2, 128).T),
        })
    return in_maps


def kernel(Q, K, V, mask, Wq, bq, Wk, bk, Wv, bv, Wo, bo, _results_hook=None):
    nc, _io = _build()
    in_maps = make_in_maps(Q, K, V, Wq, bq, Wk, bk, Wv, bv, Wo)
    res = run_bass_kernel_spmd(nc, in_maps, core_ids=list(range(NCORES)))
    if _results_hook is not None:
        _results_hook(res)
    out = np.zeros((B, S, D), np.float32)
    for core in range(NCORES):
        out[core // 4] += np.asarray(res.results[core]["out"], np.float32)
    # bv passes through the softmax average exactly; its output-space
    # contribution is the constant row bv @ Wo.T, folded in here.
    out += np.asarray(bo, np.float32) + (
        np.asarray(bv, np.float32) @ np.asarray(Wo, np.float32).T)
    return out
